# revision 49
# baseline (speedup 1.0000x reference)
"""Causal single-head attention (B=4, T=4096, D=1024, H=64) on 8 TRN2 cores.

Sharding: 2 cores per batch, queries split zig-zag for causal load balance:
  half0 (cores 0-3):  query rows [0,1024) u [3072,4096), keys all [0,4096)
  half1 (cores 4-7):  query rows [1024,3072),             keys     [0,3072)

Host passes x pre-transposed per batch (xT [D, T], bf16) so projections
stream directly with d on partitions -- no on-device transpose of x:
  pqk[h|q, t] = sum_d wkq[d, h] xT[d, t]   (lhsT = wkq d-chunk, rhs = xT)
  pv[h, t]    = sum_d wv[d, h] xT[d, t]
k/q are copied once to a staging buffer; 4 SBUF->SBUF DMAs fan out into
kT (k duplicated on both partition halves) and qT (q duplicated) so the
score matmuls can run row-packed in concurrent pairs.
v^T is transposed back to natural [t, h] via small PE matmuls into vsb
with an appended ones-column (PV matmul m=65 yields the softmax denom).
Softmax runs without max-subtraction (scores bounded ~+-2.5).
The Exp activations own the Scalar engine; most other copies go to DVE.
"""

import numpy as np
import ml_dtypes

import concourse.bass as bass
import concourse.mybir as mybir
from concourse import bacc
from concourse.tile import TileContext
from concourse.masks import make_identity
from concourse.bass_utils import run_bass_kernel_spmd

B, T, D, H = 4, 4096, 1024, 64
NCORES = 8
NQ = 2048
SCALE = 1.0 / np.sqrt(D)  # 1/32
BF16 = ml_dtypes.bfloat16

HALF_TILES = {0: [0, 1, 6, 7], 1: [2, 3, 4, 5]}

# per-half schedules: list of (out_slot, q_tile, [chunk list, diag first])
def _sched(half):
    tiles = HALF_TILES[half]
    out = []
    for slot, t in enumerate(tiles):
        diag = [4 * t + i for i in range(4)]
        fills = list(range(0, 4 * t))
        out.append((slot, t, diag + fills))
    return out

_CACHE = {}


def _build():
    if "nc" in _CACHE:
        return _CACHE["nc"]
    f32 = mybir.dt.float32
    bf16 = mybir.dt.bfloat16
    AF = mybir.ActivationFunctionType

    nc = bacc.Bacc(None, target_bir_lowering=False)
    xt_d = nc.declare_dram_parameter("xt", [D, T], bf16, isOutput=False)
    wkq_d = nc.declare_dram_parameter("wkq", [D, 128], bf16, isOutput=False)
    wv_d = nc.declare_dram_parameter("wv", [D, H], bf16, isOutput=False)
    out_d = nc.declare_dram_parameter("out", [NQ, H], f32, isOutput=True)

    with TileContext(nc) as tc:
        with (
            tc.tile_pool(name="persist", bufs=1) as pp,
            tc.tile_pool(name="work", bufs=2) as pw,
        ):
            # weights first on the DMA queue ([Wk | Wq] packed; bf16 from host)
            wkq = pp.tile([128, 1024], bf16, tag="wkq")
            nc.gpsimd.dma_start(
                out=wkq[:, :].rearrange("p (c h) -> p c h", h=128),
                in_=wkq_d[:, :].rearrange("(c p) h -> p c h", p=128))
            wv = pp.tile([128, 512], bf16, tag="wv")
            nc.gpsimd.dma_start(
                out=wv[:, :].rearrange("p (c h) -> p c h", h=64),
                in_=wv_d[:, :].rearrange("(c p) h -> p c h", p=128))

            # ---- phase 1 inputs: xT in SBUF, one tile per 512-col t-group
            # (separate tiles so group g's projection depends only on DMA g)
            # xg[p, c*512 + t'] = xT[c*128 + p, 512g + t']
            xtd_v = xt_d[:, :].rearrange("(c p) t -> p c t", p=128)
            xgs = []
            for g in range(8):
                xg = pp.tile([128, 8 * 512], bf16, tag=f"xg{g}", name=f"xg{g}")
                nc.gpsimd.dma_start(
                    out=xg[:, :].rearrange("p (c t) -> p c t", t=512),
                    in_=xtd_v[:, :, 512 * g: 512 * (g + 1)])
                xgs.append(xg)

            # ---- constants ----
            ident_f = pp.tile([128, 128], f32, tag="idf")
            make_identity(nc, ident_f[:, :])
            ident_b = pp.tile([128, 128], bf16, tag="idb")
            nc.vector.tensor_copy(ident_b[:, :], ident_f[:, :])

            # mask_big[p, g] = 1 iff g >= p + 384 (else 0)
            mask_f = pp.tile([128, 896], f32, tag="mkf")
            nc.gpsimd.memset(mask_f[:, :], 0.0)
            nc.gpsimd.affine_select(
                out=mask_f[:, :], in_=mask_f[:, :],
                compare_op=mybir.AluOpType.is_gt, fill=1.0,
                base=384, pattern=[[-1, 896]], channel_multiplier=1,
            )
            mask_b = pp.tile([128, 896], bf16, tag="mkb")
            nc.vector.tensor_copy(mask_b[:, :], mask_f[:, :])

            # preload the exp activation table off the critical path
            warm = pp.tile([1, 2], f32, tag="warm")
            nc.vector.memset(warm[:, 0:1], 0.0)
            nc.scalar.activation(warm[:, 1:2], warm[:, 0:1], AF.Exp)

            # persistent activations
            kT = pp.tile([128, T], bf16, tag="kT")       # k^T dup both halves
            qT = pp.tile([128, T], bf16, tag="qT")       # q^T dup both halves
            qkstages = [pp.tile([128, T // 2], bf16, tag=f"qks{i}", name=f"qksb{i}")
                        for i in range(2)]  # rows 0:64 k, 64:128 q
            vsb = pp.tile([128, 32 * 65], bf16, tag="vsb")
            nc.vector.memset(vsb[:, :], 1.0)             # col 64 of each chunk = 1

            # ---- phase 1: project, 8 groups of 512 t-cols ----
            with tc.tile_pool(name="ps1", bufs=2, space="PSUM") as ps1:
                for g in range(8):
                    tsl = slice(512 * g, 512 * (g + 1))
                    xg = xgs[g]
                    pqk = ps1.tile([128, 512], f32, tag="qk")
                    for dc in range(8):
                        nc.tensor.matmul(
                            pqk[:, :], lhsT=wkq[:, 128 * dc: 128 * (dc + 1)],
                            rhs=xg[:, 512 * dc: 512 * (dc + 1)],
                            start=(dc == 0), stop=(dc == 7))
                    nc.vector.tensor_copy(
                        qkstages[g // 4][:, 512 * (g % 4): 512 * (g % 4 + 1)],
                        pqk[:, :])

                    pv = ps1.tile([64, 512], f32, tag="v")
                    for dc in range(8):
                        nc.tensor.matmul(
                            pv[:, :], lhsT=wv[:, 64 * dc: 64 * (dc + 1)],
                            rhs=xg[:, 512 * dc: 512 * (dc + 1)],
                            start=(dc == 0), stop=(dc == 7))
                    vT = pw.tile([64, 512], bf16, tag="vT")
                    nc.scalar.copy(vT[:, :], pv[:, :])
                    pvn = ps1.tile([128, 256], f32, tag="vn")
                    for c in range(4):
                        nc.tensor.matmul(
                            pvn[:, 64 * c: 64 * (c + 1)],
                            lhsT=vT[0:64, 128 * c: 128 * (c + 1)],
                            rhs=ident_b[0:64, 0:64], start=True, stop=True)
                    nc.vector.tensor_copy(
                        vsb[:, 65 * 4 * g: 65 * 4 * (g + 1)].rearrange(
                            "p (c h) -> p c h", h=65)[:, :, 0:64],
                        pvn[:, :].rearrange("p (c h) -> p c h", h=64))

                    # fan out k/q (with partition duplication) per 4-group half
                    if g == 3 or g == 7:
                        qs = qkstages[g // 4]
                        csl = slice(2048 * (g // 4), 2048 * (g // 4 + 1))
                        nc.sync.dma_start(out=kT[0:64, csl], in_=qs[0:64, :])
                        nc.sync.dma_start(out=kT[64:128, csl], in_=qs[0:64, :])
                        nc.sync.dma_start(out=qT[0:64, csl], in_=qs[64:128, :])
                        nc.sync.dma_start(out=qT[64:128, csl], in_=qs[64:128, :])

            # ---- phase 2: attention, specialized per half ----
            with (
                tc.tile_pool(name="ps2", bufs=1, space="PSUM") as ps2,
                tc.tile_pool(name="ps3", bufs=1, space="PSUM") as ps3,
            ):
                # all tiles pre-allocated OUTSIDE the If (pool allocation
                # inside conditional branches breaks Tile's wait assignment)
                ps_bufs = [ps2.tile([128, 1024], f32, tag=f"sc{i}", name=f"scb{i}") for i in range(3)]
                pT_bufs = [pw.tile([128, 1024], bf16, tag=f"pT{i}", name=f"pTb{i}") for i in range(4)]
                pe2 = ps3.tile([128, 260], f32, tag="epi")
                po = ps3.tile([65, 512], f32, tag="po")
                osb_bufs = [pw.tile([65, 512], f32, tag=f"osb{i}", name=f"osbb{i}")
                            for i in range(2)]
                rc = pw.tile([128, 4], f32, tag="rc")
                outsbs = [pw.tile([128, 256], f32, tag=f"osl{s}", name=f"oslb{s}")
                          for s in range(4)]

                def phase2(half):
                    gi = 0

                    def group(qt, tsl, po, chunks, pos, n):
                        nonlocal gi
                        grp = chunks[pos: pos + 2]
                        ps = ps_bufs[gi % 3]
                        pT = pT_bufs[gi % 4]
                        gi += 1
                        c0, c1 = grp[0], grp[1]
                        nc.tensor.matmul(
                            ps[:, 0:512], lhsT=kT[0:64, 128 * c0: 128 * (c0 + 1)],
                            rhs=qT[0:64, tsl], start=True, stop=True)
                        nc.tensor.matmul(
                            ps[:, 512:1024], lhsT=kT[64:128, 128 * c1: 128 * (c1 + 1)],
                            rhs=qT[64:128, tsl], start=True, stop=True)
                        nc.scalar.activation(pT[:, :], ps[:, :], AF.Exp, scale=SCALE)
                        for jj, ch in enumerate(grp):
                            if pos + jj < 4:  # diagonal chunk: causal mask
                                delta = 128 * (pos + jj)
                                nc.vector.tensor_mul(
                                    pT[:, 512 * jj: 512 * (jj + 1)],
                                    pT[:, 512 * jj: 512 * (jj + 1)],
                                    mask_b[:, 384 - delta: 896 - delta])
                        return grp, pT

                    def pv_of(po, grp, pT, pos, n):
                        for jj, ch in enumerate(grp):
                            nc.tensor.matmul(
                                po[:, :], lhsT=vsb[:, 65 * ch: 65 * ch + 65],
                                rhs=pT[:, 512 * jj: 512 * (jj + 1)],
                                start=(pos + jj == 0), stop=(pos + jj == n - 1))

                    def epilogue(slot, po):
                        # transpose [65,512] -> [512,65], divide, store slot
                        osb = osb_bufs[slot % 2]
                        nc.vector.tensor_copy(osb[:, :], po[:, :])
                        for c in range(4):
                            nc.tensor.matmul(
                                pe2[:, 65 * c: 65 * (c + 1)],
                                lhsT=osb[0:65, 128 * c: 128 * (c + 1)],
                                rhs=ident_f[0:65, 0:65], start=True, stop=True)
                        for c in range(4):
                            nc.vector.reciprocal(rc[:, c: c + 1], pe2[:, 65 * c + 64: 65 * c + 65])
                            nc.vector.tensor_scalar_mul(
                                outsbs[slot][:, 64 * c: 64 * (c + 1)],
                                pe2[:, 65 * c: 65 * c + 64], rc[:, c: c + 1])

                    pend = None
                    for slot, qt, chunks in _sched(half):
                        n = len(chunks)
                        tsl = slice(512 * qt, 512 * (qt + 1))
                        # first group's QK/exp before the previous epilogue so
                        # the Scalar engine never waits out the PV/epilogue
                        # drain at tile boundaries
                        grp, pT = group(qt, tsl, po, chunks, 0, n)
                        if pend is not None:
                            epilogue(*pend)
                            pend = None
                        pv_of(po, grp, pT, 0, n)
                        pos = 2
                        while pos < n:
                            grp, pT = group(qt, tsl, po, chunks, pos, n)
                            pv_of(po, grp, pT, pos, n)
                            pos += 2
                        pend = (slot, po)
                    epilogue(*pend)

                pid = nc.partition_id(engines=[
                    mybir.EngineType.PE, mybir.EngineType.Activation,
                    mybir.EngineType.DVE])
                with tc.If(pid < 4) as cmp:
                    phase2(0)
                with cmp.Else():
                    phase2(1)

                # store the four slots after the If (no DMAs inside branches)
                outd_v = out_d[:, :].rearrange("(s c p) h -> s p c h", p=128, c=4)
                for s in range(4):
                    nc.sync.dma_start(
                        out=outd_v[s, :, :, :],
                        in_=outsbs[s][:, :].rearrange("p (c h) -> p c h", h=64))

    nc.compile()
    _CACHE["nc"] = nc
    return nc


def _in_maps(x, Wq, Wk, Wv):
    wkq = np.concatenate([Wk, Wq], axis=1).astype(BF16)  # [D, 128], k first
    wv = np.asarray(Wv).astype(BF16)
    maps = []
    xts = [np.ascontiguousarray(np.asarray(x[b], np.float32).T.astype(BF16))
           for b in range(B)]
    for c in range(NCORES):
        b = c % 4
        maps.append({"xt": xts[b], "wkq": wkq, "wv": wv})
    return maps


def _install_profile_shim():
    import sys, types
    import concourse.bass_utils as bu
    bu.upload_artifacts = lambda tmpdir: "local://" + tmpdir
    if "antenv.axon_hooks" in sys.modules:
        return
    mod = types.ModuleType("antenv.axon_hooks")
    holder = []
    mod.set_axon_ntff_profile_hook = holder.append
    mod.get_axon_ntff_profile_hook = lambda: holder[-1] if holder else None
    sys.modules["antenv.axon_hooks"] = mod
    import antenv
    antenv.axon_hooks = mod
    from trn_agent_boot.trn_boot import _ntff_profile_via_ctypes
    mod.set_axon_ntff_profile_hook(_ntff_profile_via_ctypes("/opt/axon/libaxon_pjrt.so"))


def kernel(x, Wq, Wk, Wv, _want_profile=False):
    if _want_profile:
        _install_profile_shim()
    nc = _build()
    maps = _in_maps(x, Wq, Wk, Wv)
    res = run_bass_kernel_spmd(nc, maps, core_ids=list(range(NCORES)),
                               trace=_want_profile)
    out = np.empty((B, T, H), np.float32)
    for c in range(NCORES):
        b, half = c % 4, c // 4
        r = np.asarray(res.results[c]["out"])
        for slot, t in enumerate(HALF_TILES[half]):
            out[b, 512 * t: 512 * (t + 1)] = r[512 * slot: 512 * (slot + 1)]
    if _want_profile:
        return out, res
    return out


# revision 50
# speedup vs baseline: 1.0168x; 1.0168x over previous
"""Causal single-head attention (B=4, T=4096, D=1024, H=64) on 8 TRN2 cores.

Sharding: 2 cores per batch, queries split zig-zag for causal load balance:
  half0 (cores 0-3):  query rows [0,1024) u [3072,4096), keys all [0,4096)
  half1 (cores 4-7):  query rows [1024,3072),             keys     [0,3072)

Host passes x pre-transposed per batch (xT [D, T], bf16) so projections
stream directly with d on partitions -- no on-device transpose of x:
  pqk[h|q, t] = sum_d wkq[d, h] xT[d, t]   (lhsT = wkq d-chunk, rhs = xT)
  pv[h, t]    = sum_d wv[d, h] xT[d, t]
k/q are copied once to a staging buffer; 4 SBUF->SBUF DMAs fan out into
kT (k duplicated on both partition halves) and qT (q duplicated) so the
score matmuls can run row-packed in concurrent pairs.
v^T is transposed back to natural [t, h] via small PE matmuls into vsb
with an appended ones-column (PV matmul m=65 yields the softmax denom).
Softmax runs without max-subtraction (scores bounded ~+-2.5).
The Exp activations own the Scalar engine; most other copies go to DVE.
"""

import numpy as np
import ml_dtypes

import concourse.bass as bass
import concourse.mybir as mybir
from concourse import bacc
from concourse.tile import TileContext
from concourse.masks import make_identity
from concourse.bass_utils import run_bass_kernel_spmd

B, T, D, H = 4, 4096, 1024, 64
NCORES = 8
NQ = 2048
SCALE = 1.0 / np.sqrt(D)  # 1/32
BF16 = ml_dtypes.bfloat16

HALF_TILES = {0: [0, 1, 6, 7], 1: [2, 3, 4, 5]}

# per-half schedules: list of (out_slot, q_tile, [chunk list, diag first])
def _sched(half):
    tiles = HALF_TILES[half]
    out = []
    for slot, t in enumerate(tiles):
        diag = [4 * t + i for i in range(4)]
        fills = list(range(0, 4 * t))
        out.append((slot, t, diag + fills))
    return out

_CACHE = {}


def _build():
    if "nc" in _CACHE:
        return _CACHE["nc"]
    f32 = mybir.dt.float32
    bf16 = mybir.dt.bfloat16
    AF = mybir.ActivationFunctionType

    nc = bacc.Bacc(None, target_bir_lowering=False)
    xt_d = nc.declare_dram_parameter("xt", [D, T], bf16, isOutput=False)
    wkq_d = nc.declare_dram_parameter("wkq", [D, 128], bf16, isOutput=False)
    wv_d = nc.declare_dram_parameter("wv", [D, H], bf16, isOutput=False)
    out_d = nc.declare_dram_parameter("out", [NQ, H], f32, isOutput=True)

    with TileContext(nc) as tc:
        with (
            tc.tile_pool(name="persist", bufs=1) as pp,
            tc.tile_pool(name="work", bufs=2) as pw,
        ):
            # weights first on the DMA queue ([Wk | Wq] packed; bf16 from host)
            wkq = pp.tile([128, 1024], bf16, tag="wkq")
            nc.gpsimd.dma_start(
                out=wkq[:, :].rearrange("p (c h) -> p c h", h=128),
                in_=wkq_d[:, :].rearrange("(c p) h -> p c h", p=128))
            wv = pp.tile([128, 512], bf16, tag="wv")
            nc.gpsimd.dma_start(
                out=wv[:, :].rearrange("p (c h) -> p c h", h=64),
                in_=wv_d[:, :].rearrange("(c p) h -> p c h", p=128))

            # ---- phase 1 inputs: xT in SBUF, one tile per 512-col t-group
            # (separate tiles so group g's projection depends only on DMA g)
            # xg[p, c*512 + t'] = xT[c*128 + p, 512g + t']
            # host pre-tiles x to the exact SBUF layout: each xg load is a
            # contiguous [128, 4096] block copy (128 x 8KB descriptors)
            xgs = []
            for g in range(8):
                xg = pp.tile([128, 8 * 512], bf16, tag=f"xg{g}", name=f"xg{g}")
                nc.gpsimd.dma_start(
                    out=xg[:, :], in_=xt_d[128 * g: 128 * (g + 1), :])
                xgs.append(xg)

            # ---- constants ----
            ident_f = pp.tile([128, 128], f32, tag="idf")
            make_identity(nc, ident_f[:, :])
            ident_b = pp.tile([128, 128], bf16, tag="idb")
            nc.vector.tensor_copy(ident_b[:, :], ident_f[:, :])

            # mask_big[p, g] = 1 iff g >= p + 384 (else 0)
            mask_f = pp.tile([128, 896], f32, tag="mkf")
            nc.gpsimd.memset(mask_f[:, :], 0.0)
            nc.gpsimd.affine_select(
                out=mask_f[:, :], in_=mask_f[:, :],
                compare_op=mybir.AluOpType.is_gt, fill=1.0,
                base=384, pattern=[[-1, 896]], channel_multiplier=1,
            )
            mask_b = pp.tile([128, 896], bf16, tag="mkb")
            nc.vector.tensor_copy(mask_b[:, :], mask_f[:, :])

            # preload the exp activation table off the critical path
            warm = pp.tile([1, 2], f32, tag="warm")
            nc.vector.memset(warm[:, 0:1], 0.0)
            nc.scalar.activation(warm[:, 1:2], warm[:, 0:1], AF.Exp)

            # persistent activations
            kT = pp.tile([128, T], bf16, tag="kT")       # k^T dup both halves
            qT = pp.tile([128, T], bf16, tag="qT")       # q^T dup both halves
            qkstages = [pp.tile([128, T // 2], bf16, tag=f"qks{i}", name=f"qksb{i}")
                        for i in range(2)]  # rows 0:64 k, 64:128 q
            vsb = pp.tile([128, 32 * 65], bf16, tag="vsb")
            nc.vector.memset(vsb[:, :], 1.0)             # col 64 of each chunk = 1

            # ---- phase 1: project, 8 groups of 512 t-cols ----
            with tc.tile_pool(name="ps1", bufs=2, space="PSUM") as ps1:
                for g in range(8):
                    tsl = slice(512 * g, 512 * (g + 1))
                    xg = xgs[g]
                    pqk = ps1.tile([128, 512], f32, tag="qk")
                    for dc in range(8):
                        nc.tensor.matmul(
                            pqk[:, :], lhsT=wkq[:, 128 * dc: 128 * (dc + 1)],
                            rhs=xg[:, 512 * dc: 512 * (dc + 1)],
                            start=(dc == 0), stop=(dc == 7))
                    nc.vector.tensor_copy(
                        qkstages[g // 4][:, 512 * (g % 4): 512 * (g % 4 + 1)],
                        pqk[:, :])

                    pv = ps1.tile([64, 512], f32, tag="v")
                    for dc in range(8):
                        nc.tensor.matmul(
                            pv[:, :], lhsT=wv[:, 64 * dc: 64 * (dc + 1)],
                            rhs=xg[:, 512 * dc: 512 * (dc + 1)],
                            start=(dc == 0), stop=(dc == 7))
                    vT = pw.tile([64, 512], bf16, tag="vT")
                    nc.scalar.copy(vT[:, :], pv[:, :])
                    pvn = ps1.tile([128, 256], f32, tag="vn")
                    for c in range(4):
                        nc.tensor.matmul(
                            pvn[:, 64 * c: 64 * (c + 1)],
                            lhsT=vT[0:64, 128 * c: 128 * (c + 1)],
                            rhs=ident_b[0:64, 0:64], start=True, stop=True)
                    nc.vector.tensor_copy(
                        vsb[:, 65 * 4 * g: 65 * 4 * (g + 1)].rearrange(
                            "p (c h) -> p c h", h=65)[:, :, 0:64],
                        pvn[:, :].rearrange("p (c h) -> p c h", h=64))

                    # fan out k/q (with partition duplication) per 4-group half
                    if g == 3 or g == 7:
                        qs = qkstages[g // 4]
                        csl = slice(2048 * (g // 4), 2048 * (g // 4 + 1))
                        nc.sync.dma_start(out=kT[0:64, csl], in_=qs[0:64, :])
                        nc.sync.dma_start(out=kT[64:128, csl], in_=qs[0:64, :])
                        nc.sync.dma_start(out=qT[0:64, csl], in_=qs[64:128, :])
                        nc.sync.dma_start(out=qT[64:128, csl], in_=qs[64:128, :])

            # ---- phase 2: attention, specialized per half ----
            with (
                tc.tile_pool(name="ps2", bufs=1, space="PSUM") as ps2,
                tc.tile_pool(name="ps3", bufs=1, space="PSUM") as ps3,
            ):
                # all tiles pre-allocated OUTSIDE the If (pool allocation
                # inside conditional branches breaks Tile's wait assignment)
                ps_bufs = [ps2.tile([128, 1024], f32, tag=f"sc{i}", name=f"scb{i}") for i in range(3)]
                pT_bufs = [pw.tile([128, 1024], bf16, tag=f"pT{i}", name=f"pTb{i}") for i in range(4)]
                pe2 = ps3.tile([128, 260], f32, tag="epi")
                po = ps3.tile([65, 512], f32, tag="po")
                osb_bufs = [pw.tile([65, 512], f32, tag=f"osb{i}", name=f"osbb{i}")
                            for i in range(2)]
                rc = pw.tile([128, 4], f32, tag="rc")
                outsbs = [pw.tile([128, 256], f32, tag=f"osl{s}", name=f"oslb{s}")
                          for s in range(4)]

                def phase2(half):
                    gi = 0

                    def group(qt, tsl, po, chunks, pos, n):
                        nonlocal gi
                        grp = chunks[pos: pos + 2]
                        ps = ps_bufs[gi % 3]
                        pT = pT_bufs[gi % 4]
                        gi += 1
                        c0, c1 = grp[0], grp[1]
                        nc.tensor.matmul(
                            ps[:, 0:512], lhsT=kT[0:64, 128 * c0: 128 * (c0 + 1)],
                            rhs=qT[0:64, tsl], start=True, stop=True)
                        nc.tensor.matmul(
                            ps[:, 512:1024], lhsT=kT[64:128, 128 * c1: 128 * (c1 + 1)],
                            rhs=qT[64:128, tsl], start=True, stop=True)
                        nc.scalar.activation(pT[:, :], ps[:, :], AF.Exp, scale=SCALE)
                        for jj, ch in enumerate(grp):
                            if pos + jj < 4:  # diagonal chunk: causal mask
                                delta = 128 * (pos + jj)
                                nc.vector.tensor_mul(
                                    pT[:, 512 * jj: 512 * (jj + 1)],
                                    pT[:, 512 * jj: 512 * (jj + 1)],
                                    mask_b[:, 384 - delta: 896 - delta])
                        return grp, pT

                    def pv_of(po, grp, pT, pos, n):
                        for jj, ch in enumerate(grp):
                            nc.tensor.matmul(
                                po[:, :], lhsT=vsb[:, 65 * ch: 65 * ch + 65],
                                rhs=pT[:, 512 * jj: 512 * (jj + 1)],
                                start=(pos + jj == 0), stop=(pos + jj == n - 1))

                    def epilogue(slot, po):
                        # transpose [65,512] -> [512,65], divide, store slot
                        osb = osb_bufs[slot % 2]
                        nc.vector.tensor_copy(osb[:, :], po[:, :])
                        for c in range(4):
                            nc.tensor.matmul(
                                pe2[:, 65 * c: 65 * (c + 1)],
                                lhsT=osb[0:65, 128 * c: 128 * (c + 1)],
                                rhs=ident_f[0:65, 0:65], start=True, stop=True)
                        for c in range(4):
                            nc.vector.reciprocal(rc[:, c: c + 1], pe2[:, 65 * c + 64: 65 * c + 65])
                            nc.vector.tensor_scalar_mul(
                                outsbs[slot][:, 64 * c: 64 * (c + 1)],
                                pe2[:, 65 * c: 65 * c + 64], rc[:, c: c + 1])

                    pend = None
                    for slot, qt, chunks in _sched(half):
                        n = len(chunks)
                        tsl = slice(512 * qt, 512 * (qt + 1))
                        # first group's QK/exp before the previous epilogue so
                        # the Scalar engine never waits out the PV/epilogue
                        # drain at tile boundaries
                        grp, pT = group(qt, tsl, po, chunks, 0, n)
                        if pend is not None:
                            epilogue(*pend)
                            pend = None
                        pv_of(po, grp, pT, 0, n)
                        pos = 2
                        while pos < n:
                            grp, pT = group(qt, tsl, po, chunks, pos, n)
                            pv_of(po, grp, pT, pos, n)
                            pos += 2
                        pend = (slot, po)
                    epilogue(*pend)

                pid = nc.partition_id(engines=[
                    mybir.EngineType.PE, mybir.EngineType.Activation,
                    mybir.EngineType.DVE])
                with tc.If(pid < 4) as cmp:
                    phase2(0)
                with cmp.Else():
                    phase2(1)

                # store the four slots after the If (no DMAs inside branches)
                outd_v = out_d[:, :].rearrange("(s c p) h -> s p c h", p=128, c=4)
                for s in range(4):
                    nc.sync.dma_start(
                        out=outd_v[s, :, :, :],
                        in_=outsbs[s][:, :].rearrange("p (c h) -> p c h", h=64))

    nc.compile()
    _CACHE["nc"] = nc
    return nc


def _in_maps(x, Wq, Wk, Wv):
    wkq = np.concatenate([Wk, Wq], axis=1).astype(BF16)  # [D, 128], k first
    wv = np.asarray(Wv).astype(BF16)
    maps = []
    xts = []
    for b in range(B):
        xt = np.asarray(x[b], np.float32).T.astype(BF16)     # [D, T]
        xt4 = xt.reshape(8, 128, 8, 512)                     # [dc, p, g, t]
        xts.append(np.ascontiguousarray(
            xt4.transpose(2, 1, 0, 3).reshape(1024, 4096)))  # [g*p, dc*t]
    for c in range(NCORES):
        b = c % 4
        maps.append({"xt": xts[b], "wkq": wkq, "wv": wv})
    return maps


def _install_profile_shim():
    import sys, types
    import concourse.bass_utils as bu
    bu.upload_artifacts = lambda tmpdir: "local://" + tmpdir
    if "antenv.axon_hooks" in sys.modules:
        return
    mod = types.ModuleType("antenv.axon_hooks")
    holder = []
    mod.set_axon_ntff_profile_hook = holder.append
    mod.get_axon_ntff_profile_hook = lambda: holder[-1] if holder else None
    sys.modules["antenv.axon_hooks"] = mod
    import antenv
    antenv.axon_hooks = mod
    from trn_agent_boot.trn_boot import _ntff_profile_via_ctypes
    mod.set_axon_ntff_profile_hook(_ntff_profile_via_ctypes("/opt/axon/libaxon_pjrt.so"))


def kernel(x, Wq, Wk, Wv, _want_profile=False):
    if _want_profile:
        _install_profile_shim()
    nc = _build()
    maps = _in_maps(x, Wq, Wk, Wv)
    res = run_bass_kernel_spmd(nc, maps, core_ids=list(range(NCORES)),
                               trace=_want_profile)
    out = np.empty((B, T, H), np.float32)
    for c in range(NCORES):
        b, half = c % 4, c // 4
        r = np.asarray(res.results[c]["out"])
        for slot, t in enumerate(HALF_TILES[half]):
            out[b, 512 * t: 512 * (t + 1)] = r[512 * slot: 512 * (slot + 1)]
    if _want_profile:
        return out, res
    return out
